# revision 16
# baseline (speedup 1.0000x reference)
"""Content-based (Bahdanau-style) attention kernel for Trainium2.

Computes, per batch b:
    e      = tanh(keys @ W_s.T + q[b] @ W_h.T + b)     # [S, H]
    energy = e @ v                                      # [S]
    w      = softmax(energy)                            # [S]
    ctx    = w @ keys                                   # [H]

Full shapes: keys [32, 4096, 512], q [1, 32, 512], W* [512, 512].
Sharding: data-parallel over the batch dim -> 4 batches per core on 8
NeuronCores, weights replicated, no collectives. Output gathered on host.

Host-side prep (layout/dtype marshalling only — all matmuls stay on
device): keys are passed twice, natural [t, h] and transposed [h, t],
both fp16, so the device never runs PE transposes for the big tensor;
W_s/W_h are passed pre-transposed fp16; q is passed as fp16 columns.
fp16 (10-bit mantissa) keeps the final rel err ~3e-4, well under the
2e-2 gate; all PSUM accumulation stays fp32.

Per-core pipeline per 512-token block (PE does only real contractions):
  - 2 big DMAs: kt_all [128, 4x512] fp16 (keysT tiles) + kn_all
    [128, 4x512] fp16 (natural tiles).
  - PE: 16 fp16 MMs accumulate pre.T [o, t] in PSUM.
  - ScalarE: tanh with fused per-partition bias (q@W_h.T + b, computed
    once on device in the preamble) -> et fp16.
  - PE: 4 fp16 MMs dot et with v -> energy [1, t] in PSUM.
  - ScalarE: Exp -> w row fp32, with fused denominator accumulation.
  - PE: 4 tiny transposes -> w columns fp16, then 4 *column-tiled*
    fp16 MMs (tile_position=(0,32*t4)) that run concurrently in
    disjoint 32-column groups of the PE array, accumulating 4 partial
    context rows (PSUM partitions 0/32/64/96) across the whole batch.
    The w-dependent tail of block tb is emitted after the main MMs of
    block tb+1, so the PE never stalls on the Exp chain.
  - Batch end: one DVE copy PSUM->SBUF, one fp32r selector MM sums the
    4 partial rows, reciprocal + scale, out-DMA.
Softmax max-subtraction is skipped deliberately: energies are ~N(0,0.7)
(max |energy| ~ 3.5 over this dataset), exp() cannot overflow fp32.
"""

import numpy as np
from contextlib import ExitStack

import concourse.bass as bass
import concourse.tile as tile
from concourse import mybir
from concourse.bass_utils import run_bass_kernel_spmd
from concourse.masks import make_identity

H = 512
S = 4096
B = 32
N_CORES = 8
LOCAL_B = B // N_CORES
FP = mybir.dt.float32
FR = mybir.dt.float32r
F16 = mybir.dt.float16
TBLK = 512  # tokens per inner block

MAX_WAITS = 1


def split_sync_waits(nc):
    """This container's walrus rejects >1 sem-wait per instruction (all
    encodings); split overflow waits onto carrier nops placed just before
    the offender (same engine, so ordering is preserved)."""
    n_split = 0
    for f in nc.m.functions:
        for bb in f.blocks:
            snapshot = list(bb.instructions)
            inserts = []
            for idx, ins in enumerate(snapshot):
                w = ins.sync_info.on_wait if ins.sync_info else None
                if w and len(w) > MAX_WAITS:
                    chunks = [w[i:i + MAX_WAITS] for i in range(0, len(w), MAX_WAITS)]
                    ins.sync_info.on_wait = chunks[-1]
                    nops = []
                    for j, ch in enumerate(chunks[:-1]):
                        nop = mybir.InstNoOp(
                            name=f"waitsplit-{ins.name}-{j}", ins=[], outs=[])
                        nop.engine = ins.engine
                        nop.sync_info = mybir.SyncInfo(on_wait=ch, on_update=[])
                        nops.append(nop)
                    inserts.append((idx, nops))
                    n_split += 1
            for idx, nops in reversed(inserts):
                for nop in reversed(nops):
                    bb.instructions.insert(idx, nop)
    return n_split


def build(local_b=LOCAL_B, s=S, repeat=1, split_waits=True,
          pre_bufs=3, kbufs=4, et_bufs=8, energy_bufs=2,
          col_ctx=True, defer=2, kn_eng="gpsimd", kt_eng="sync"):
    """Build the per-core Bass program. `repeat` re-runs the whole body
    (identical outputs) for wall-clock differencing in test harnesses.
    col_ctx: run the 4 per-block context MMs concurrently in 4 PE
    column groups (tile_position) instead of serial accumulation.
    defer: 0 = in-order; 1 = emit the w-dependent tail of block tb
    after block tb+1's main matmuls; 2 = additionally defer the energy
    MMs + Exp by one block (every cross-engine dependency then has a
    full block of slack before the PE needs its result)."""
    nc = bass.Bass()
    ktd = nc.declare_dram_parameter("ktd", [local_b * H, s], F16, isOutput=False)
    knd = nc.declare_dram_parameter("knd", [local_b * s, H], F16, isOutput=False)
    wsT_d = nc.declare_dram_parameter("wsT", [H, H], F16, isOutput=False)
    whT_d = nc.declare_dram_parameter("whT", [H, H], F16, isOutput=False)
    qT_d = nc.declare_dram_parameter("qT", [128, 4 * local_b], F16, isOutput=False)
    v_d = nc.declare_dram_parameter("v16", [128, 4], F16, isOutput=False)
    b_d = nc.declare_dram_parameter("b16", [1, H], F16, isOutput=False)
    out_d = nc.declare_dram_parameter("out", [local_b, H], FP, isOutput=True)

    n_tblk = s // TBLK
    LB = local_b

    def eng(name):
        return {"sync": nc.sync, "gpsimd": nc.gpsimd, "vector": nc.vector,
                "scalar": nc.scalar}[name]

    with ExitStack() as ctx:
        tc = ctx.enter_context(tile.TileContext(nc))
        const_pool = ctx.enter_context(tc.tile_pool(name="const", bufs=1))
        kt_pool = ctx.enter_context(tc.tile_pool(name="kt", bufs=kbufs))
        kn_pool = ctx.enter_context(tc.tile_pool(name="kn", bufs=kbufs))
        et_pool = ctx.enter_context(tc.tile_pool(name="et", bufs=et_bufs))
        small_pool = ctx.enter_context(tc.tile_pool(name="small", bufs=4))
        psum_pre = ctx.enter_context(tc.tile_pool(name="ppre", bufs=pre_bufs, space="PSUM"))
        psum_en = ctx.enter_context(tc.tile_pool(name="pen", bufs=energy_bufs, space="PSUM"))
        psum_misc = ctx.enter_context(tc.tile_pool(name="pmisc", bufs=1, space="PSUM"))
        psum_ctxp = ctx.enter_context(tc.tile_pool(name="pctx", bufs=1, space="PSUM"))

        ident = const_pool.tile([128, 128], FP)
        make_identity(nc, ident)
        ones_row = const_pool.tile([1, 128], F16)
        nc.vector.memset(ones_row, 1.0)
        sel4 = None
        if col_ctx:
            # selector column: 1.0 at partitions {0,32,64,96}
            sel4f = const_pool.tile([128, 1], FP, tag="sel4f")
            nc.vector.memset(sel4f, 0.0)
            for t4 in range(4):
                nc.vector.memset(sel4f[32 * t4:32 * t4 + 1, :], 1.0)
            sel4t = const_pool.tile([128, 1], FP, tag="sel4")
            nc.vector.tensor_copy(sel4t.bitcast(FR), sel4f)
            sel4 = sel4t.bitcast(FR)

        # replicated weights / small tensors
        wsT = []
        whT = []
        for ic in range(4):
            t = const_pool.tile([128, H], F16, tag=f"wsT{ic}")
            nc.sync.dma_start(out=t, in_=wsT_d[ic * 128:(ic + 1) * 128, :])
            wsT.append(t)
            t = const_pool.tile([128, H], F16, tag=f"whT{ic}")
            nc.sync.dma_start(out=t, in_=whT_d[ic * 128:(ic + 1) * 128, :])
            whT.append(t)
        qT = const_pool.tile([128, 4 * LB], F16)
        nc.sync.dma_start(out=qT, in_=qT_d[:, :])
        v_sb = const_pool.tile([128, 4], F16)
        nc.sync.dma_start(out=v_sb, in_=v_d[:, :])
        b_sb = const_pool.tile([1, H], F16)
        nc.sync.dma_start(out=b_sb, in_=b_d[:, :])

        # qwh[b, o] = q[b] @ W_h.T + b  -> per-(oc, batch) bias columns
        pq = psum_pre.tile([128, TBLK], FP, tag="pre")
        for ic in range(4):
            nc.tensor.matmul(pq[:LB, :H], lhsT=qT[:, ic * LB:(ic + 1) * LB],
                             rhs=whT[ic], start=(ic == 0), stop=False)
        nc.tensor.matmul(pq[:LB, :H], lhsT=ones_row[:, :LB], rhs=b_sb,
                         start=False, stop=True)
        qwh_sb = const_pool.tile([LB, H], FP)
        nc.scalar.copy(qwh_sb, pq[:LB, :H])
        qwhbT = const_pool.tile([128, 4 * LB], FP)
        for oc in range(4):
            pt = psum_misc.tile([128, 4], FP, tag="mix")
            nc.tensor.transpose(pt[:, :LB], qwh_sb[:, oc * 128:(oc + 1) * 128],
                                ident[:LB, :LB])
            nc.vector.tensor_copy(qwhbT[:, oc * LB:(oc + 1) * LB], pt[:, :LB])

        if col_ctx:
            # zero the context bank once: unwritten rows must read as 0.0
            pz = psum_ctxp.tile([128, TBLK], FP, tag="ctx4")
            nc.vector.memset(pz, 0.0)

        for rep in range(repeat):
            for lb in range(LB):
                denom = small_pool.tile([1, n_tblk], FP, tag="denom")
                if col_ctx:
                    pctx4 = psum_ctxp.tile([128, TBLK], FP, tag="ctx4")
                else:
                    pctx4 = psum_ctxp.tile([1, H], FP, tag="ctx4")

                def emit_tail(w_row, kn_all, tb):
                    pwT = psum_misc.tile([128, 4], FP, tag="mix")
                    for t4 in range(4):
                        nc.tensor.transpose(pwT[:, t4:t4 + 1],
                                            w_row[:, t4 * 128:(t4 + 1) * 128],
                                            ident[:1, :1])
                    w_col = small_pool.tile([128, 4], F16, tag="wcol")
                    nc.vector.tensor_copy(w_col, pwT[:, :4])
                    for t4 in range(4):
                        if col_ctx:
                            nc.tensor.matmul(
                                pctx4[32 * t4:32 * t4 + 1, :],
                                lhsT=w_col[:, t4:t4 + 1],
                                rhs=kn_all[:, t4 * H:(t4 + 1) * H],
                                start=(tb == 0), stop=(tb == n_tblk - 1),
                                tile_position=(0, 32 * t4))
                        else:
                            nc.tensor.matmul(
                                pctx4, lhsT=w_col[:, t4:t4 + 1],
                                rhs=kn_all[:, t4 * H:(t4 + 1) * H],
                                start=(tb == 0 and t4 == 0),
                                stop=(tb == n_tblk - 1 and t4 == 3))

                pending = None
                pending_b = None
                pending_c = None
                for tb in range(n_tblk):
                    # keysT tiles: kt_all[p, c*512+t] = keysT[lb, c*128+p, tb*512+t]
                    kt_all = kt_pool.tile([128, 4 * TBLK], F16, tag="kt")
                    eng(kt_eng).dma_start(
                        out=kt_all.rearrange("p (c t) -> p c t", c=4),
                        in_=ktd[lb * H:(lb + 1) * H, tb * TBLK:(tb + 1) * TBLK]
                        .rearrange("(c p) t -> p c t", p=128))
                    # natural tiles: kn_all[p, c*512+h] = keys[lb, tb*512+c*128+p, h]
                    kn_all = kn_pool.tile([128, 4 * H], F16, tag="kn")
                    base = lb * s + tb * TBLK
                    eng(kn_eng).dma_start(
                        out=kn_all.rearrange("p (c h) -> p c h", c=4),
                        in_=knd[base:base + TBLK, :]
                        .rearrange("(c p) h -> p c h", p=128))

                    ets = []
                    for oc in range(4):
                        ppre = psum_pre.tile([128, TBLK], FP, tag="pre")
                        for ic in range(4):
                            nc.tensor.matmul(
                                ppre, lhsT=wsT[ic][:, oc * 128:(oc + 1) * 128],
                                rhs=kt_all[:, ic * TBLK:(ic + 1) * TBLK],
                                start=(ic == 0), stop=(ic == 3))
                        et = et_pool.tile([128, TBLK], F16, tag="et")
                        nc.scalar.activation(
                            et, ppre, mybir.ActivationFunctionType.Tanh,
                            bias=qwhbT[:, oc * LB + lb: oc * LB + lb + 1],
                            scale=1.0)
                        ets.append(et)

                    def emit_energy(ets_, tb_):
                        pe_energy = psum_en.tile([1, TBLK], FP, tag="energy")
                        for oc in range(4):
                            nc.tensor.matmul(pe_energy, lhsT=v_sb[:, oc:oc + 1],
                                             rhs=ets_[oc], start=(oc == 0),
                                             stop=(oc == 3))
                        w_row = small_pool.tile([1, TBLK], FP, tag="wrow")
                        nc.scalar.activation(w_row, pe_energy,
                                             mybir.ActivationFunctionType.Exp,
                                             accum_out=denom[:, tb_:tb_ + 1])
                        return w_row

                    if defer == 2:
                        new_c = None
                        if pending_b is not None:
                            b_ets, b_kn, b_tb = pending_b
                            b_w = emit_energy(b_ets, b_tb)
                            new_c = (b_w, b_kn, b_tb)
                        if pending_c is not None:
                            emit_tail(*pending_c)
                        pending_c = new_c
                        pending_b = (ets, kn_all, tb)
                    elif defer == 1:
                        w_row = emit_energy(ets, tb)
                        if pending is not None:
                            emit_tail(*pending)
                        pending = (w_row, kn_all, tb)
                        if tb == n_tblk - 1:
                            emit_tail(*pending)
                            pending = None
                    else:
                        w_row = emit_energy(ets, tb)
                        emit_tail(w_row, kn_all, tb)
                if defer == 2:
                    b_ets, b_kn, b_tb = pending_b
                    b_w = emit_energy(b_ets, b_tb)
                    if pending_c is not None:
                        emit_tail(*pending_c)
                    emit_tail(b_w, b_kn, b_tb)
                    pending_b = None
                    pending_c = None

                dsum = small_pool.tile([1, 1], FP, tag="dsum")
                nc.vector.tensor_reduce(dsum, denom, axis=mybir.AxisListType.X,
                                        op=mybir.AluOpType.add)
                rec = small_pool.tile([1, 1], FP, tag="rec")
                nc.vector.reciprocal(rec, dsum)
                if col_ctx:
                    ctx4_sb = small_pool.tile([128, TBLK], FP, tag="ctx4sb")
                    nc.vector.tensor_copy(ctx4_sb.bitcast(FR), pctx4)
                    pcs = psum_misc.tile([1, H], FP, tag="mix")
                    nc.tensor.matmul(pcs, lhsT=sel4,
                                     rhs=ctx4_sb.bitcast(FR),
                                     start=True, stop=True)
                    ctx_row = small_pool.tile([1, H], FP, tag="ctxrow")
                    nc.vector.tensor_scalar_mul(ctx_row, pcs, rec)
                else:
                    ctx_row = small_pool.tile([1, H], FP, tag="ctxrow")
                    nc.vector.tensor_scalar_mul(ctx_row, pctx4, rec)
                nc.sync.dma_start(out=out_d[lb:lb + 1, :], in_=ctx_row)

    if split_waits:
        split_sync_waits(nc)
    return nc


def prepare_in_maps(encoder_outputs, decoder_h_t, W_h, W_s, v, b):
    """Host-side layout/dtype marshalling -> per-core DRAM input dicts."""
    keys16 = np.asarray(encoder_outputs, dtype=np.float16)          # [B, S, H]
    ktd16 = np.ascontiguousarray(keys16.transpose(0, 2, 1))         # [B, H, S]
    q = np.asarray(decoder_h_t, dtype=np.float32)[0]                # [B, H]
    wsT16 = np.ascontiguousarray(np.asarray(W_s, dtype=np.float32).T
                                 .astype(np.float16))               # [h, o]
    whT16 = np.ascontiguousarray(np.asarray(W_h, dtype=np.float32).T
                                 .astype(np.float16))
    v16 = np.ascontiguousarray(
        np.asarray(v, dtype=np.float32).reshape(4, 128).T.astype(np.float16))
    b16 = np.asarray(b, dtype=np.float32).reshape(1, H).astype(np.float16)

    in_maps = []
    for c in range(N_CORES):
        lo, hi = c * LOCAL_B, (c + 1) * LOCAL_B
        qc = q[lo:hi]                                               # [LB, H]
        # qT[p, ic*LB + b] = qc[b, ic*128 + p]
        qT = np.ascontiguousarray(
            qc.reshape(LOCAL_B, 4, 128).transpose(2, 1, 0)
            .reshape(128, 4 * LOCAL_B).astype(np.float16))
        in_maps.append({
            "ktd": ktd16[lo:hi].reshape(LOCAL_B * H, S),
            "knd": keys16[lo:hi].reshape(LOCAL_B * S, H),
            "wsT": wsT16,
            "whT": whT16,
            "qT": qT,
            "v16": v16,
            "b16": b16,
        })
    return in_maps


_NC_CACHE = {}


def _get_nc(repeat=1):
    if repeat not in _NC_CACHE:
        _NC_CACHE[repeat] = build(repeat=repeat)
    return _NC_CACHE[repeat]


def kernel(encoder_outputs, decoder_h_t, W_h, W_s, v, b):
    in_maps = prepare_in_maps(encoder_outputs, decoder_h_t, W_h, W_s, v, b)
    nc = _get_nc()
    res = run_bass_kernel_spmd(nc, in_maps, core_ids=list(range(N_CORES)))
    out = np.concatenate([res.results[c]["out"] for c in range(N_CORES)], axis=0)
    return out.reshape(B, 1, H).astype(np.float32)


# revision 22
# speedup vs baseline: 3.4074x; 3.4074x over previous
"""Content-based (Bahdanau-style) attention kernel for Trainium2.

Computes, per batch b:
    e      = tanh(keys @ W_s.T + q[b] @ W_h.T + b)     # [S, H]
    energy = e @ v                                      # [S]
    w      = softmax(energy)                            # [S]
    ctx    = w @ keys                                   # [H]

Full shapes: keys [32, 4096, 512], q [1, 32, 512], W* [512, 512].
Sharding: data-parallel over the batch dim -> 4 batches per core on 8
NeuronCores, weights replicated, no collectives. Output gathered on host.

Host-side prep (layout/dtype marshalling only — all matmuls stay on
device): keys are passed twice, natural [t, h] and transposed [h, t],
both fp16, so the device never runs PE transposes for the big tensor;
W_s/W_h are passed pre-transposed fp16; q is passed as fp16 columns.
fp16 (10-bit mantissa) keeps the final rel err ~3e-4, well under the
2e-2 gate; all PSUM accumulation stays fp32.

Per-core pipeline per 512-token block (PE does only real contractions):
  - 2 big DMAs: kt_all [128, 4x512] fp16 (keysT tiles) + kn_all
    [128, 4x512] fp16 (natural tiles).
  - PE: 16 fp16 MMs accumulate pre.T [o, t] in PSUM.
  - ScalarE: tanh with fused per-partition bias (q@W_h.T + b, computed
    once on device in the preamble) -> et fp16.
  - PE: 4 fp16 MMs dot et with v -> energy [1, t] in PSUM.
  - ScalarE: Exp -> w row fp32, with fused denominator accumulation.
  - PE: 4 tiny transposes -> w columns fp16, then 4 *column-tiled*
    fp16 MMs (tile_position=(0,32*t4)) that run concurrently in
    disjoint 32-column groups of the PE array, accumulating 4 partial
    context rows (PSUM partitions 0/32/64/96) across the whole batch.
    The w-dependent tail of block tb is emitted after the main MMs of
    block tb+1, so the PE never stalls on the Exp chain.
  - Batch end: one DVE copy PSUM->SBUF, one fp32r selector MM sums the
    4 partial rows, reciprocal + scale, out-DMA.
Softmax max-subtraction is skipped deliberately: energies are ~N(0,0.7)
(max |energy| ~ 3.5 over this dataset), exp() cannot overflow fp32.
"""

import numpy as np
from contextlib import ExitStack

import concourse.bass as bass
import concourse.tile as tile
from concourse import mybir
from concourse.bass_utils import run_bass_kernel_spmd
from concourse.masks import make_identity

H = 512
S = 4096
B = 32
N_CORES = 8
LOCAL_B = B // N_CORES
FP = mybir.dt.float32
FR = mybir.dt.float32r
F16 = mybir.dt.float16
TBLK = 512  # tokens per inner block

MAX_WAITS = 1


def split_sync_waits(nc):
    """This container's walrus rejects >1 sem-wait per instruction (all
    encodings); split overflow waits onto carrier nops placed just before
    the offender (same engine, so ordering is preserved)."""
    n_split = 0
    for f in nc.m.functions:
        for bb in f.blocks:
            snapshot = list(bb.instructions)
            inserts = []
            for idx, ins in enumerate(snapshot):
                w = ins.sync_info.on_wait if ins.sync_info else None
                if w and len(w) > MAX_WAITS:
                    chunks = [w[i:i + MAX_WAITS] for i in range(0, len(w), MAX_WAITS)]
                    ins.sync_info.on_wait = chunks[-1]
                    nops = []
                    for j, ch in enumerate(chunks[:-1]):
                        nop = mybir.InstNoOp(
                            name=f"waitsplit-{ins.name}-{j}", ins=[], outs=[])
                        nop.engine = ins.engine
                        nop.sync_info = mybir.SyncInfo(on_wait=ch, on_update=[])
                        nops.append(nop)
                    inserts.append((idx, nops))
                    n_split += 1
            for idx, nops in reversed(inserts):
                for nop in reversed(nops):
                    bb.instructions.insert(idx, nop)
    return n_split


def build(local_b=LOCAL_B, s=S, repeat=1, split_waits=True,
          pre_bufs=3, kbufs=4, et_bufs=8, energy_bufs=2,
          col_ctx=True, energy_col=True, defer=2,
          kn_eng="gpsimd", kt_eng="sync"):
    """Build the per-core Bass program. `repeat` re-runs the whole body
    (identical outputs) for wall-clock differencing in test harnesses.
    col_ctx: run the 4 per-block context MMs concurrently in 4 PE
    column groups (tile_position) instead of serial accumulation.
    defer: 0 = in-order; 1 = emit the w-dependent tail of block tb
    after block tb+1's main matmuls; 2 = additionally defer the energy
    MMs + Exp by one block (every cross-engine dependency then has a
    full block of slack before the PE needs its result)."""
    nc = bass.Bass()
    ktd = nc.declare_dram_parameter("ktd", [local_b * H, s], F16, isOutput=False)
    knd = nc.declare_dram_parameter("knd", [local_b * s, H], F16, isOutput=False)
    wsT_d = nc.declare_dram_parameter("wsT", [H, H], F16, isOutput=False)
    whT_d = nc.declare_dram_parameter("whT", [H, H], F16, isOutput=False)
    qT_d = nc.declare_dram_parameter("qT", [128, 4 * local_b], F16, isOutput=False)
    v_d = nc.declare_dram_parameter("v16", [128, 4], F16, isOutput=False)
    b_d = nc.declare_dram_parameter("b16", [1, H], F16, isOutput=False)
    out_d = nc.declare_dram_parameter("out", [local_b, H], FP, isOutput=True)

    n_tblk = s // TBLK
    LB = local_b

    def eng(name):
        return {"sync": nc.sync, "gpsimd": nc.gpsimd, "vector": nc.vector,
                "scalar": nc.scalar}[name]

    with ExitStack() as ctx:
        tc = ctx.enter_context(tile.TileContext(nc))
        const_pool = ctx.enter_context(tc.tile_pool(name="const", bufs=1))
        kt_pool = ctx.enter_context(tc.tile_pool(name="kt", bufs=kbufs))
        kn_pool = ctx.enter_context(tc.tile_pool(name="kn", bufs=kbufs))
        et_pool = ctx.enter_context(tc.tile_pool(name="et", bufs=et_bufs))
        small_pool = ctx.enter_context(tc.tile_pool(name="small", bufs=4))
        psum_pre = ctx.enter_context(tc.tile_pool(name="ppre", bufs=pre_bufs, space="PSUM"))
        psum_en = ctx.enter_context(tc.tile_pool(name="pen", bufs=energy_bufs, space="PSUM"))
        psum_misc = ctx.enter_context(tc.tile_pool(name="pmisc", bufs=1, space="PSUM"))
        psum_ctxp = ctx.enter_context(tc.tile_pool(name="pctx", bufs=1, space="PSUM"))
        psum_es = None
        if energy_col:
            psum_es = ctx.enter_context(tc.tile_pool(name="pes", bufs=1, space="PSUM"))

        ident = const_pool.tile([128, 128], FP)
        make_identity(nc, ident)
        ones_row = const_pool.tile([1, 128], F16)
        nc.vector.memset(ones_row, 1.0)
        sel4 = None
        if col_ctx or energy_col:
            # selector column: 1.0 at partitions {0,32,64,96}
            sel4f = const_pool.tile([128, 1], FP, tag="sel4f")
            nc.vector.memset(sel4f, 0.0)
            for t4 in range(4):
                nc.vector.memset(sel4f[32 * t4:32 * t4 + 1, :], 1.0)
            sel4t = const_pool.tile([128, 1], FP, tag="sel4")
            nc.vector.tensor_copy(sel4t.bitcast(FR), sel4f)
            sel4 = sel4t.bitcast(FR)

        # replicated weights / small tensors
        wsT = []
        whT = []
        for ic in range(4):
            t = const_pool.tile([128, H], F16, tag=f"wsT{ic}")
            nc.sync.dma_start(out=t, in_=wsT_d[ic * 128:(ic + 1) * 128, :])
            wsT.append(t)
            t = const_pool.tile([128, H], F16, tag=f"whT{ic}")
            nc.sync.dma_start(out=t, in_=whT_d[ic * 128:(ic + 1) * 128, :])
            whT.append(t)
        qT = const_pool.tile([128, 4 * LB], F16)
        nc.sync.dma_start(out=qT, in_=qT_d[:, :])
        v_sb = const_pool.tile([128, 4], F16)
        nc.sync.dma_start(out=v_sb, in_=v_d[:, :])
        b_sb = const_pool.tile([1, H], F16)
        nc.sync.dma_start(out=b_sb, in_=b_d[:, :])

        # qwh[b, o] = q[b] @ W_h.T + b  -> per-(oc, batch) bias columns
        pq = psum_pre.tile([128, TBLK], FP, tag="pre")
        for ic in range(4):
            nc.tensor.matmul(pq[:LB, :H], lhsT=qT[:, ic * LB:(ic + 1) * LB],
                             rhs=whT[ic], start=(ic == 0), stop=False)
        nc.tensor.matmul(pq[:LB, :H], lhsT=ones_row[:, :LB], rhs=b_sb,
                         start=False, stop=True)
        qwh_sb = const_pool.tile([LB, H], FP)
        nc.scalar.copy(qwh_sb, pq[:LB, :H])
        qwhbT = const_pool.tile([128, 4 * LB], FP)
        for oc in range(4):
            pt = psum_misc.tile([128, 4], FP, tag="mix")
            nc.tensor.transpose(pt[:, :LB], qwh_sb[:, oc * 128:(oc + 1) * 128],
                                ident[:LB, :LB])
            nc.vector.tensor_copy(qwhbT[:, oc * LB:(oc + 1) * LB], pt[:, :LB])

        if col_ctx:
            # zero the context bank once: unwritten rows must read as 0.0
            pz = psum_ctxp.tile([128, TBLK], FP, tag="ctx4")
            nc.vector.memset(pz, 0.0)
        if energy_col:
            # zero both energy banks once (same reason)
            for _ in range(energy_bufs):
                pz = psum_en.tile([128, TBLK], FP, tag="energy")
                nc.vector.memset(pz, 0.0)

        for rep in range(repeat):
            for lb in range(LB):
                denom = small_pool.tile([1, n_tblk], FP, tag="denom")
                if col_ctx:
                    pctx4 = psum_ctxp.tile([128, TBLK], FP, tag="ctx4")
                else:
                    pctx4 = psum_ctxp.tile([1, H], FP, tag="ctx4")

                def emit_tail(w_row, kn_all, tb):
                    pwT = psum_misc.tile([128, 4], FP, tag="mix")
                    for t4 in range(4):
                        nc.tensor.transpose(pwT[:, t4:t4 + 1],
                                            w_row[:, t4 * 128:(t4 + 1) * 128],
                                            ident[:1, :1])
                    w_col = small_pool.tile([128, 4], F16, tag="wcol")
                    nc.vector.tensor_copy(w_col, pwT[:, :4])
                    for t4 in range(4):
                        if col_ctx:
                            nc.tensor.matmul(
                                pctx4[32 * t4:32 * t4 + 1, :],
                                lhsT=w_col[:, t4:t4 + 1],
                                rhs=kn_all[:, t4 * H:(t4 + 1) * H],
                                start=(tb == 0), stop=(tb == n_tblk - 1),
                                tile_position=(0, 32 * t4))
                        else:
                            nc.tensor.matmul(
                                pctx4, lhsT=w_col[:, t4:t4 + 1],
                                rhs=kn_all[:, t4 * H:(t4 + 1) * H],
                                start=(tb == 0 and t4 == 0),
                                stop=(tb == n_tblk - 1 and t4 == 3))

                pending = None
                pending_b = None
                pending_c = None
                for tb in range(n_tblk):
                    # keysT tiles: kt_all[p, c*512+t] = keysT[lb, c*128+p, tb*512+t]
                    kt_all = kt_pool.tile([128, 4 * TBLK], F16, tag="kt")
                    eng(kt_eng).dma_start(
                        out=kt_all.rearrange("p (c t) -> p c t", c=4),
                        in_=ktd[lb * H:(lb + 1) * H, tb * TBLK:(tb + 1) * TBLK]
                        .rearrange("(c p) t -> p c t", p=128))
                    # natural tiles: kn_all[p, c*512+h] = keys[lb, tb*512+c*128+p, h]
                    kn_all = kn_pool.tile([128, 4 * H], F16, tag="kn")
                    base = lb * s + tb * TBLK
                    eng(kn_eng).dma_start(
                        out=kn_all.rearrange("p (c h) -> p c h", c=4),
                        in_=knd[base:base + TBLK, :]
                        .rearrange("(c p) h -> p c h", p=128))

                    ets = []
                    for oc in range(4):
                        ppre = psum_pre.tile([128, TBLK], FP, tag="pre")
                        for ic in range(4):
                            nc.tensor.matmul(
                                ppre, lhsT=wsT[ic][:, oc * 128:(oc + 1) * 128],
                                rhs=kt_all[:, ic * TBLK:(ic + 1) * TBLK],
                                start=(ic == 0), stop=(ic == 3))
                        et = et_pool.tile([128, TBLK], F16, tag="et")
                        nc.scalar.activation(
                            et, ppre, mybir.ActivationFunctionType.Tanh,
                            bias=qwhbT[:, oc * LB + lb: oc * LB + lb + 1],
                            scale=1.0)
                        ets.append(et)

                    def emit_energy(ets_, tb_):
                        if energy_col:
                            # 4 concurrent col-group MMs -> partial rows at
                            # partitions {0,32,64,96}, then one fp32r
                            # selector MM sums them.
                            e4 = psum_en.tile([128, TBLK], FP, tag="energy")
                            for oc in range(4):
                                nc.tensor.matmul(
                                    e4[32 * oc:32 * oc + 1, :],
                                    lhsT=v_sb[:, oc:oc + 1], rhs=ets_[oc],
                                    start=True, stop=True,
                                    tile_position=(0, 32 * oc))
                            e4sb = small_pool.tile([128, TBLK], FP, tag="e4sb")
                            nc.vector.tensor_copy(e4sb.bitcast(FR), e4)
                            pe_energy = psum_es.tile([1, TBLK], FP, tag="esum")
                            nc.tensor.matmul(pe_energy, lhsT=sel4,
                                             rhs=e4sb.bitcast(FR),
                                             start=True, stop=True)
                        else:
                            pe_energy = psum_en.tile([1, TBLK], FP, tag="energy")
                            for oc in range(4):
                                nc.tensor.matmul(pe_energy,
                                                 lhsT=v_sb[:, oc:oc + 1],
                                                 rhs=ets_[oc], start=(oc == 0),
                                                 stop=(oc == 3))
                        w_row = small_pool.tile([1, TBLK], FP, tag="wrow")
                        nc.scalar.activation(w_row, pe_energy,
                                             mybir.ActivationFunctionType.Exp,
                                             accum_out=denom[:, tb_:tb_ + 1])
                        return w_row

                    if defer == 2:
                        new_c = None
                        if pending_b is not None:
                            b_ets, b_kn, b_tb = pending_b
                            b_w = emit_energy(b_ets, b_tb)
                            new_c = (b_w, b_kn, b_tb)
                        if pending_c is not None:
                            emit_tail(*pending_c)
                        pending_c = new_c
                        pending_b = (ets, kn_all, tb)
                    elif defer == 1:
                        w_row = emit_energy(ets, tb)
                        if pending is not None:
                            emit_tail(*pending)
                        pending = (w_row, kn_all, tb)
                        if tb == n_tblk - 1:
                            emit_tail(*pending)
                            pending = None
                    else:
                        w_row = emit_energy(ets, tb)
                        emit_tail(w_row, kn_all, tb)
                if defer == 2:
                    b_ets, b_kn, b_tb = pending_b
                    b_w = emit_energy(b_ets, b_tb)
                    if pending_c is not None:
                        emit_tail(*pending_c)
                    emit_tail(b_w, b_kn, b_tb)
                    pending_b = None
                    pending_c = None

                dsum = small_pool.tile([1, 1], FP, tag="dsum")
                nc.vector.tensor_reduce(dsum, denom, axis=mybir.AxisListType.X,
                                        op=mybir.AluOpType.add)
                rec = small_pool.tile([1, 1], FP, tag="rec")
                nc.vector.reciprocal(rec, dsum)
                if col_ctx:
                    ctx4_sb = small_pool.tile([128, TBLK], FP, tag="ctx4sb")
                    nc.vector.tensor_copy(ctx4_sb.bitcast(FR), pctx4)
                    pcs = psum_misc.tile([1, H], FP, tag="mix")
                    nc.tensor.matmul(pcs, lhsT=sel4,
                                     rhs=ctx4_sb.bitcast(FR),
                                     start=True, stop=True)
                    ctx_row = small_pool.tile([1, H], FP, tag="ctxrow")
                    nc.vector.tensor_scalar_mul(ctx_row, pcs, rec)
                else:
                    ctx_row = small_pool.tile([1, H], FP, tag="ctxrow")
                    nc.vector.tensor_scalar_mul(ctx_row, pctx4, rec)
                nc.sync.dma_start(out=out_d[lb:lb + 1, :], in_=ctx_row)

    if split_waits:
        split_sync_waits(nc)
    return nc


def prepare_in_maps(encoder_outputs, decoder_h_t, W_h, W_s, v, b):
    """Host-side layout/dtype marshalling -> per-core DRAM input dicts."""
    keys16 = np.asarray(encoder_outputs, dtype=np.float16)          # [B, S, H]
    ktd16 = np.ascontiguousarray(keys16.transpose(0, 2, 1))         # [B, H, S]
    q = np.asarray(decoder_h_t, dtype=np.float32)[0]                # [B, H]
    wsT16 = np.ascontiguousarray(np.asarray(W_s, dtype=np.float32).T
                                 .astype(np.float16))               # [h, o]
    whT16 = np.ascontiguousarray(np.asarray(W_h, dtype=np.float32).T
                                 .astype(np.float16))
    v16 = np.ascontiguousarray(
        np.asarray(v, dtype=np.float32).reshape(4, 128).T.astype(np.float16))
    b16 = np.asarray(b, dtype=np.float32).reshape(1, H).astype(np.float16)

    in_maps = []
    for c in range(N_CORES):
        lo, hi = c * LOCAL_B, (c + 1) * LOCAL_B
        qc = q[lo:hi]                                               # [LB, H]
        # qT[p, ic*LB + b] = qc[b, ic*128 + p]
        qT = np.ascontiguousarray(
            qc.reshape(LOCAL_B, 4, 128).transpose(2, 1, 0)
            .reshape(128, 4 * LOCAL_B).astype(np.float16))
        in_maps.append({
            "ktd": ktd16[lo:hi].reshape(LOCAL_B * H, S),
            "knd": keys16[lo:hi].reshape(LOCAL_B * S, H),
            "wsT": wsT16,
            "whT": whT16,
            "qT": qT,
            "v16": v16,
            "b16": b16,
        })
    return in_maps


_NC_CACHE = {}


def _get_nc(repeat=1):
    if repeat not in _NC_CACHE:
        _NC_CACHE[repeat] = build(repeat=repeat)
    return _NC_CACHE[repeat]


def kernel(encoder_outputs, decoder_h_t, W_h, W_s, v, b):
    in_maps = prepare_in_maps(encoder_outputs, decoder_h_t, W_h, W_s, v, b)
    nc = _get_nc()
    res = run_bass_kernel_spmd(nc, in_maps, core_ids=list(range(N_CORES)))
    out = np.concatenate([res.results[c]["out"] for c in range(N_CORES)], axis=0)
    return out.reshape(B, 1, H).astype(np.float32)
